# revision 17
# baseline (speedup 1.0000x reference)
"""Trainium2 Bass kernel for DifferentiablePointMassSimulator.

Math: the 2-D point-mass scan is reformulated in polar velocity coordinates.
With v = r*e^{i*theta}, a_t = DT*thrust, b_t = DT*torque:
    v' = e^{i*theta} * (r + a + i*b)
so the radius obeys a scalar recurrence independent of the angle:
    m_{t+1} = (m_t + (a^2+b^2)_t) + (2*a_t)*r_t,   r_t = sqrt(m_t)
and the angle increment delta_t = atan2(b_t, r_t + a_t) is computed post-hoc
from the radius sequence with the quarter-angle identity
    delta = 4*atan( b / (h + w1) ),  w1 = u + r',  u = r_t + a_t,  r' = r_{t+1}
    h = sqrt(2 * r' * w1)
whose atan argument always lies in [-1, 1] (ScalarE Arctan domain).
Near the delta ~ +-pi line (u < 0, |b| << |u|) the direct w1 = u + r' suffers
catastrophic cancellation; there we use the exact rationalization
    w1 = b^2 / (r' - u)        (since r'^2 - u^2 = b^2)
selected with copy_predicated on (u < 0).
theta_t = theta0 + cumsum(delta) via tensor_tensor_scan.  sin/cos via the
magic-constant round-to-nearest range reduction: with y = theta*2/pi (turns),
f = y - ((y + 1.5*2^23) - 1.5*2^23) lies in [-0.5, 0.5], and
sin(2*pi*f) = sin(theta) via the ScalarE Sin table (cos via y + 0.25).
Reciprocals are exp(-ln(x)) on ScalarE (custom DVE ops and the Reciprocal /
Rsqrt tables are unavailable in this toolchain).
Positions: pos_{t+1} = pos_t + DT*(v_t + v_{t+1})/2 exactly, so with
vxs_t = DT*vx_out[t]:
    px_out[t] = Cx_t - 0.5*vxs_t,  Cx = scan(+, vxs, init = px0 + DT*vx0/2).

Sharding: pure data parallel, batch 16384 -> 8 cores x 2048; on-core layout
batch = 128 partitions x 16 columns (b_local = p*16 + col).
"""

import sys

sys.path.insert(0, "/opt/trn_rl_repo")

import numpy as np

import concourse.bass as bass
import concourse.mybir as mybir
from concourse.tile import TileContext

DT = 1.0 / 30.0
P = 128          # partitions
NB = 16          # batch columns per partition
H = 256          # horizon
HP = H + 1
S = 8            # state dim
BC = P * NB      # batch per core (2048)
NCORES = 8
B = BC * NCORES

F32 = mybir.dt.float32
PI = float(np.pi)
TWO_PI = float(2.0 * np.pi)

_BUILT = None


def build_nc(fixups=True):
    Alu = mybir.AluOpType
    AF = mybir.ActivationFunctionType

    nc = bass.Bass()
    ist = nc.dram_tensor("initial_state", [BC, S], F32, kind="ExternalInput")
    act = nc.dram_tensor("actions", [BC, H, 2], F32, kind="ExternalInput")
    traj = nc.dram_tensor("traj", [BC, H, S], F32, kind="ExternalOutput")

    ist_r = ist.rearrange("(p q) s -> p (q s)", p=P)       # (128, 128)
    act_r = act.rearrange("(p q) h a -> p (q h a)", p=P)   # (128, 8192)
    traj_r = traj.rearrange("(p q) h s -> p (q h s)", p=P)  # (128, 32768)

    v = nc.vector
    g = nc.gpsimd
    sc = nc.scalar
    sy = nc.sync

    with TileContext(nc) as tc:
        with tc.tile_pool(name="pers", bufs=1) as pp, \
                tc.tile_pool(name="outc", bufs=2) as op:
            RP = pp.tile([P, NB * HP], F32, tag="RP")      # r_k at slot k
            A2 = pp.tile([P, NB * H], F32, tag="A2")       # 2*DT*thrust
            BQ = pp.tile([P, NB * H], F32, tag="BQ")       # DT*torque
            CARR = pp.tile([P, NB * H], F32, tag="CARR")   # a^2+b^2
            IS = pp.tile([P, NB * S], F32, tag="IS")
            # big tmps: 3 explicit rotating slots
            S1 = pp.tile([P, NB * H], F32, tag="S1")
            S2 = pp.tile([P, NB * H], F32, tag="S2")
            S3 = pp.tile([P, NB * H], F32, tag="S3")
            # small state tiles, packed into one allocation
            SMALL = pp.tile([P, NB * 12], F32, tag="SMALL")
            M = SMALL[:, 0 * NB:1 * NB]
            T1 = SMALL[:, 1 * NB:2 * NB]
            GA = SMALL[:, 2 * NB:3 * NB]   # scan scratch half 0
            GB = SMALL[:, 3 * NB:4 * NB]   # scan scratch half 1
            Q0 = SMALL[:, 4 * NB:5 * NB]
            A0 = SMALL[:, 5 * NB:6 * NB]
            KX = SMALL[:, 6 * NB:7 * NB]
            KY = SMALL[:, 7 * NB:8 * NB]
            W10 = SMALL[:, 8 * NB:9 * NB]
            RMU0 = SMALL[:, 9 * NB:10 * NB]
            MSK0 = SMALL[:, 10 * NB:11 * NB]

            # multi-dim views
            IS3 = IS.rearrange("p (b s) -> p b s", b=NB)
            RP3 = RP.rearrange("p (b k) -> p b k", b=NB)
            A23 = A2.rearrange("p (b t) -> p b t", b=NB)
            BQ3 = BQ.rearrange("p (b t) -> p b t", b=NB)
            C3 = CARR.rearrange("p (b t) -> p b t", b=NB)

            px0 = IS3[:, :, 0]
            py0 = IS3[:, :, 1]
            vx0 = IS3[:, :, 2]
            vy0 = IS3[:, :, 3]

            # ---------------- phase 0: loads + precompute ----------------
            sy.dma_start(out=IS[:], in_=ist_r[:])

            # actions -> A2, BQ, CARR (two 2MB chunks; squares on ScalarE)
            for hb in range(2):
                chunk = pp.tile([P, 8 * H * 2], F32, tag="S1" if hb == 0 else "CH1")
                for dq in range(2):
                    sy.dma_start(
                        out=chunk[:, dq * 2048:(dq + 1) * 2048],
                        in_=act_r[:, hb * 4096 + dq * 2048:hb * 4096 + (dq + 1) * 2048],
                    )
                ch = chunk.rearrange("p (b t a) -> p b t a", b=8, t=H)
                thr = ch[:, :, :, 0]
                tor = ch[:, :, :, 1]
                bsl = slice(hb * 8, (hb + 1) * 8)
                v.tensor_scalar(A23[:, bsl, :], thr, 2.0 * DT, None, Alu.mult)
                v.tensor_scalar(BQ3[:, bsl, :], tor, DT, None, Alu.mult)
                sq = pp.tile([P, 8 * H], F32, tag="S3")
                sq3 = sq.rearrange("p (b t) -> p b t", b=8)
                sc.activation(sq3, thr, AF.Square, scale=DT)   # (DT*T)^2
                sq2 = pp.tile([P, 8 * H], F32, tag="S1" if hb == 1 else "S2")
                sq23 = sq2.rearrange("p (b t) -> p b t", b=8)
                sc.activation(sq23, tor, AF.Square, scale=DT)  # (DT*Q)^2
                v.tensor_add(C3[:, bsl, :], sq3, sq23)

            # r0, m0
            sc.activation(GA, vx0, AF.Square)
            sc.activation(GB, vy0, AF.Square)
            v.tensor_add(M, GA, GB)                      # m0 = r0^2
            sc.activation(RP3[:, :, 0], M, AF.Sqrt)      # r0
            r0 = RP3[:, :, 0]

            # theta0/4 prep: w10 = r0 + vx0, rationalized to vy0^2/(r0 - vx0)
            # when vx0 < 0.  All reciprocals are deferred to the ln/exp table
            # section after the scan (no custom DVE ops available).
            v.tensor_add(W10, r0, vx0)                   # w10 direct
            v.tensor_sub(RMU0, r0, vx0)                  # r0 - vx0
            MSK0i = MSK0.bitcast(mybir.dt.int32)
            v.tensor_scalar(MSK0i, vx0, 0.0, None, Alu.is_lt)  # mask vx0 < 0

            # pos cumsum seeds
            v.scalar_tensor_tensor(KX, vx0, DT / 2.0, px0, Alu.mult, Alu.add)
            v.scalar_tensor_tensor(KY, vy0, DT / 2.0, py0, Alu.mult, Alu.add)

            # ---------------- phase 1: radius scan ----------------
            # m' = (m + c_t) + (2 a_t) * r_t ; r_{t+1} = sqrt(m')
            # two staggered halves so ScalarE sqrt overlaps VectorE updates
            halves = [slice(0, 8), slice(8, 16)]
            Mh = [M[:, 0:8], M[:, 8:16]]
            T1h = [T1[:, 0:8], T1[:, 8:16]]
            Gh = [GA[:, 0:8], GB[:, 0:8]]
            # Pre-allocate the first two output-chunk staging tiles so their
            # extras channels (which depend only on initial_state) can be
            # broadcast by gpsimd DURING the radius scan.  One extras op is
            # interleaved every 16 scan steps: gpsimd's T1 stream stays ahead
            # of the DVE/Act chain (scan consumes T1 at ~600ns/step; gpsimd
            # produces at ~340ns/step + 131ns/step of amortized extras).
            CB = 4                       # batch-columns per output chunk
            CW = CB * H
            OUTC_pre0 = op.tile([P, CB * H * S], F32, tag="OUTC")
            OUTC_pre1 = op.tile([P, CB * H * S], F32, tag="OUTC")
            OUTC_pre = [OUTC_pre0, OUTC_pre1]

            def emit_extras(ch, k, outc):
                out_ap = bass.AP(
                    outc.tensor, 4 + k, [[CB * H * S, P], [H * S, CB], [S, H]]
                )
                in_ap = bass.AP(
                    IS.tensor, ch * CB * S + 4 + k,
                    [[NB * S, P], [S, CB], [0, H]],
                )
                g.tensor_copy(out_ap, in_ap)

            # T1_t = m_t + c_t is computed at the END of iteration t-1 (just
            # after m_t is produced) so that on the in-order DVE queue it
            # executes during the wait for the sqrt semaphore instead of
            # delaying the chain-critical G/M' ops.
            for hf in (0, 1):
                v.tensor_add(T1h[hf], Mh[hf], C3[:, halves[hf], 0])
            for t in range(H):
                if t % 16 == 0 and 16 <= t <= 128:
                    idx = t // 16 - 1  # 0..7 -> (ch, k)
                    emit_extras(idx // 4, idx % 4, OUTC_pre[idx // 4])
                for hf in (0, 1):
                    v.tensor_mul(Gh[hf], A23[:, halves[hf], t], RP3[:, halves[hf], t])
                    v.tensor_add(Mh[hf], T1h[hf], Gh[hf])
                    sc.activation(RP3[:, halves[hf], t + 1], Mh[hf], AF.Sqrt)
                if t + 1 < H:
                    for hf in (0, 1):
                        v.tensor_add(T1h[hf], Mh[hf], C3[:, halves[hf], t + 1])

            # ---------------- phase 2: angles, velocities, positions ------
            Rsh = RP3[:, :, 0:H]     # r_t
            Rpo = RP3[:, :, 1:HP]    # r_{t+1}
            S1_3 = S1.rearrange("p (b t) -> p b t", b=NB)
            S2_3 = S2.rearrange("p (b t) -> p b t", b=NB)
            S3_3 = S3.rearrange("p (b t) -> p b t", b=NB)

            # theta0 chain (small; DVE reciprocal keeps Act in the sqrt table)
            v.reciprocal(GA, RMU0)                        # 1/(r0-vx0)
            v.tensor_mul(GB, vy0, GA)
            v.tensor_mul(GB, vy0, GB)                     # alt0
            v.copy_predicated(W10, MSK0i, GB)             # w10
            v.tensor_mul(GB, r0, W10)
            sc.activation(GB, GB, AF.Sqrt, scale=2.0)     # h0 = sqrt(2*w2)
            v.tensor_add(GB, GB, W10)                     # den0
            v.reciprocal(GA, GB)
            v.tensor_mul(Q0, vy0, GA)                     # q0

            # Phase 2 proper runs in TWO COLUMN HALVES so the first half's
            # output DMAs overlap the second half's compute.  Per half:
            # A-section (select-free atan2 identity):
            #   delta = sign(b) * (pi/2 - 2*atan(u / (r' + |b|))),  u = r_t + a
            # whose atan argument is always within [-1, 1] and well-conditioned
            # in every quadrant (no predicated rationalization); then trig via
            # magic-constant range reduction, then the C-section + DMA out.
            MAGIC = float(1.5 * 2 ** 23)
            INV_HPI = float(2.0 / np.pi)                  # turns = Theta*4/(2*pi)
            BQi = BQ[:].bitcast(mybir.dt.int32)
            HW2 = 8 * H                                   # floats per half
            VXY = pp.tile([P, 4 * CW], F32, tag="CH1")    # vxs/vys ping-pong
            for hs in range(2):
                colh = slice(hs * 8, (hs + 1) * 8)
                fsl = slice(hs * HW2, (hs + 1) * HW2)
                S1h, S2h, S3h = S1[:, fsl], S2[:, fsl], S3[:, fsl]
                S1h3 = S1_3[:, colh, :]
                S2h3 = S2_3[:, colh, :]
                S3h3 = S3_3[:, colh, :]
                A23h = A23[:, colh, :]
                Rsh_h = RP3[:, colh, 0:H]
                Rpo_h = RP3[:, colh, 1:HP]
                BQhi = BQ[:, fsl].bitcast(mybir.dt.int32)
                # A-section: all-DVE up to qu (reciprocal instead of Ln/Exp,
                # so Act needs only one sqrt->trig table switch in the kernel)
                v.scalar_tensor_tensor(S1h3, A23h, 0.5, Rsh_h, Alu.mult, Alu.add)
                S2hi = S2h.bitcast(mybir.dt.int32)
                v.tensor_scalar(S2hi, BQhi, 0x7FFFFFFF, None, Alu.bitwise_and)
                v.tensor_add(S2h3, S2h3, Rpo_h)           # den = r' + |b|
                v.reciprocal(S3h, S2h)                    # rden
                v.tensor_mul(S2h3, S1h3, S3h3)            # qu = u*rden
                # trig table phase; |qu| <= 1 + O(eps) needs no clamp
                if hs == 0:
                    sc.activation(A0, Q0, AF.Arctan)      # theta0/4
                sc.activation(S1h3, S2h3, AF.Arctan)      # At
                v.tensor_scalar(S1h, S1h, -0.5, PI / 8.0, Alu.mult, Alu.add)
                S1hi = S1h.bitcast(mybir.dt.int32)
                S3hi = S3h.bitcast(mybir.dt.int32)
                v.tensor_scalar(S3hi, BQhi, -0x80000000, None, Alu.bitwise_and)
                v.tensor_tensor(S1hi, S1hi, S3hi, Alu.bitwise_or)  # delta/4
                for b in range(8):
                    gb = hs * 8 + b
                    bs = slice(gb * H, (gb + 1) * H)
                    v.tensor_tensor_scan(
                        S3[:, bs], S1[:, bs], S1[:, bs],
                        initial=A0[:, gb:gb + 1], op0=Alu.add, op1=Alu.bypass,
                    )                                      # Theta
                v.tensor_scalar(S2h, S3h, INV_HPI, None, Alu.mult)       # yS
                v.tensor_scalar(S1h, S2h, MAGIC, -MAGIC, Alu.add, Alu.add)
                v.tensor_sub(S2h, S2h, S1h)               # fS
                sc.activation(S2h, S2h, AF.Sin, scale=TWO_PI)   # sin
                v.tensor_scalar(S1h, S3h, INV_HPI, 0.25, Alu.mult, Alu.add)
                v.tensor_scalar(S3h, S1h, MAGIC, -MAGIC, Alu.add, Alu.add)
                v.tensor_sub(S1h, S1h, S3h)               # fC
                sc.activation(S1h, S1h, AF.Sin, scale=TWO_PI)   # cos
                # C-section for this half's two chunks
                for chl in range(2):
                    ch = hs * 2 + chl
                    cols = slice(ch * CB, (ch + 1) * CB)
                    OUTC = OUTC_pre[ch] if ch < 2 else op.tile(
                        [P, CB * H * S], F32, tag="OUTC"
                    )
                    OC4 = OUTC.rearrange("p (b t s) -> p b t s", b=CB, t=H)
                    base = chl * 2 * CW
                    vxs = VXY[:, base:base + CW]
                    vys = VXY[:, base + CW:base + 2 * CW]
                    vxs3 = vxs.rearrange("p (b t) -> p b t", b=CB)
                    vys3 = vys.rearrange("p (b t) -> p b t", b=CB)
                    Rpo_c = RP3[:, cols, 1:HP]
                    sin_c = S2_3[:, cols, :]
                    cos_c = S1_3[:, cols, :]
                    g.tensor_mul(OC4[:, :, :, 2], Rpo_c, cos_c)           # vx
                    g.tensor_mul(OC4[:, :, :, 3], Rpo_c, sin_c)           # vy
                    v.scalar_tensor_tensor(vxs3, cos_c, DT, Rpo_c, Alu.mult, Alu.mult)
                    v.scalar_tensor_tensor(vys3, sin_c, DT, Rpo_c, Alu.mult, Alu.mult)
                    for j in range(CB):
                        b = ch * CB + j
                        js = slice(j * H, (j + 1) * H)
                        v.tensor_tensor_scan(
                            OC4[:, j, :, 0], vxs[:, js], vxs[:, js],
                            initial=KX[:, b:b + 1], op0=Alu.add, op1=Alu.bypass,
                        )
                        v.tensor_tensor_scan(
                            OC4[:, j, :, 1], vys[:, js], vys[:, js],
                            initial=KY[:, b:b + 1], op0=Alu.add, op1=Alu.bypass,
                        )
                    v.scalar_tensor_tensor(
                        OC4[:, :, :, 0], vxs3, -0.5, OC4[:, :, :, 0], Alu.mult, Alu.add
                    )
                    v.scalar_tensor_tensor(
                        OC4[:, :, :, 1], vys3, -0.5, OC4[:, :, :, 1], Alu.mult, Alu.add
                    )
                    # extra columns broadcast from initial_state (gpsimd);
                    # chunks 0 and 1 were prefilled during the radius scan
                    if ch >= 2:
                        for k in range(4):
                            emit_extras(ch, k, OUTC)
                    hw = CB * H * S // 2
                    base_o = ch * CB * H * S
                    sy.dma_start(
                        out=traj_r[:, base_o:base_o + hw], in_=OUTC[:, 0:hw]
                    )
                    sy.dma_start(
                        out=traj_r[:, base_o + hw:base_o + 2 * hw],
                        in_=OUTC[:, hw:2 * hw],
                    )

    nc.finalize()
    if fixups:
        _split_multi_waits(nc)
    return nc


def _split_multi_waits(nc):
    """This toolchain's walrus embeds at most ONE sync-wait per instruction.
    Move all but the last wait of any multi-wait instruction onto NoOps
    inserted just before it (same engine, program order preserved).  Also
    drop the tail EVENT_SEMAPHORE_RANGE_CLEAR InstISA, whose raw encoding
    this walrus rejects ("ISA wrong length")."""
    n = 0
    for fn in nc.m.functions:
        for bb in fn.blocks:
            idx = 0
            while idx < len(bb.instructions):
                inst = bb.instructions[idx]
                if (
                    isinstance(inst, mybir.InstISA)
                    and getattr(inst, "op_name", "") == "EVENT_SEMAPHORE_RANGE_CLEAR"
                ):
                    del bb.instructions[idx]
                    continue
                si = getattr(inst, "sync_info", None)
                if si is not None and si.on_wait and len(si.on_wait) >= 2:
                    extra = list(si.on_wait[:-1])
                    keep = list(si.on_wait[-1:])
                    for w in extra:
                        nop = mybir.InstNoOp(
                            name=f"{inst.name}_wsplit{n}", ins=[], outs=[]
                        )
                        n += 1
                        nop.engine = inst.engine
                        nop.sync_info = mybir.SyncInfo(on_wait=[w], on_update=[])
                        bb.instructions.insert(idx, nop)
                        idx += 1
                    inst.sync_info = mybir.SyncInfo(
                        on_wait=keep, on_update=list(si.on_update)
                    )
                idx += 1
    return nc


def _get_built():
    global _BUILT
    if _BUILT is None:
        _BUILT = build_nc()
    return _BUILT


def kernel(initial_state: np.ndarray, actions: np.ndarray) -> np.ndarray:
    from concourse.bass_utils import run_bass_kernel_spmd

    nc = _get_built()
    in_maps = []
    for c in range(NCORES):
        sl = slice(c * BC, (c + 1) * BC)
        in_maps.append(
            {
                "initial_state": np.ascontiguousarray(initial_state[sl]),
                "actions": np.ascontiguousarray(actions[sl]),
            }
        )
    res = run_bass_kernel_spmd(nc, in_maps, core_ids=list(range(NCORES)))
    out = np.concatenate([r["traj"] for r in res.results], axis=0)
    return out



# revision 18
# speedup vs baseline: 1.0283x; 1.0283x over previous
"""Trainium2 Bass kernel for DifferentiablePointMassSimulator.

Math: the 2-D point-mass scan is reformulated in polar velocity coordinates.
With v = r*e^{i*theta}, a_t = DT*thrust, b_t = DT*torque:
    v' = e^{i*theta} * (r + a + i*b)
so the radius obeys a scalar recurrence independent of the angle:
    m_{t+1} = (m_t + (a^2+b^2)_t) + (2*a_t)*r_t,   r_t = sqrt(m_t)
and the angle increment delta_t = atan2(b_t, r_t + a_t) is computed post-hoc
from the radius sequence with the quarter-angle identity
    delta = 4*atan( b / (h + w1) ),  w1 = u + r',  u = r_t + a_t,  r' = r_{t+1}
    h = sqrt(2 * r' * w1)
whose atan argument always lies in [-1, 1] (ScalarE Arctan domain).
Near the delta ~ +-pi line (u < 0, |b| << |u|) the direct w1 = u + r' suffers
catastrophic cancellation; there we use the exact rationalization
    w1 = b^2 / (r' - u)        (since r'^2 - u^2 = b^2)
selected with copy_predicated on (u < 0).
theta_t = theta0 + cumsum(delta) via tensor_tensor_scan.  sin/cos via the
magic-constant round-to-nearest range reduction: with y = theta*2/pi (turns),
f = y - ((y + 1.5*2^23) - 1.5*2^23) lies in [-0.5, 0.5], and
sin(2*pi*f) = sin(theta) via the ScalarE Sin table (cos via y + 0.25).
Reciprocals are exp(-ln(x)) on ScalarE (custom DVE ops and the Reciprocal /
Rsqrt tables are unavailable in this toolchain).
Positions: pos_{t+1} = pos_t + DT*(v_t + v_{t+1})/2 exactly, so with
vxs_t = DT*vx_out[t]:
    px_out[t] = Cx_t - 0.5*vxs_t,  Cx = scan(+, vxs, init = px0 + DT*vx0/2).

Sharding: pure data parallel, batch 16384 -> 8 cores x 2048; on-core layout
batch = 128 partitions x 16 columns (b_local = p*16 + col).
"""

import sys

sys.path.insert(0, "/opt/trn_rl_repo")

import numpy as np

import concourse.bass as bass
import concourse.mybir as mybir
from concourse.tile import TileContext

DT = 1.0 / 30.0
P = 128          # partitions
NB = 16          # batch columns per partition
H = 256          # horizon
HP = H + 1
S = 8            # state dim
BC = P * NB      # batch per core (2048)
NCORES = 8
B = BC * NCORES

F32 = mybir.dt.float32
PI = float(np.pi)
TWO_PI = float(2.0 * np.pi)

_BUILT = None


def build_nc(fixups=True):
    Alu = mybir.AluOpType
    AF = mybir.ActivationFunctionType

    nc = bass.Bass()
    ist = nc.dram_tensor("initial_state", [BC, S], F32, kind="ExternalInput")
    act = nc.dram_tensor("actions", [BC, H, 2], F32, kind="ExternalInput")
    traj = nc.dram_tensor("traj", [BC, H, S], F32, kind="ExternalOutput")

    ist_r = ist.rearrange("(p q) s -> p (q s)", p=P)       # (128, 128)
    act_r = act.rearrange("(p q) h a -> p (q h a)", p=P)   # (128, 8192)
    traj_r = traj.rearrange("(p q) h s -> p (q h s)", p=P)  # (128, 32768)

    v = nc.vector
    g = nc.gpsimd
    sc = nc.scalar
    sy = nc.sync

    with TileContext(nc) as tc:
        with tc.tile_pool(name="pers", bufs=1) as pp, \
                tc.tile_pool(name="outc", bufs=2) as op:
            RP = pp.tile([P, NB * HP], F32, tag="RP")      # r_k at slot k
            A2 = pp.tile([P, NB * H], F32, tag="A2")       # 2*DT*thrust
            BQ = pp.tile([P, NB * H], F32, tag="BQ")       # DT*torque
            CARR = pp.tile([P, NB * H], F32, tag="CARR")   # a^2+b^2
            IS = pp.tile([P, NB * S], F32, tag="IS")
            # big tmps: 3 explicit rotating slots
            S1 = pp.tile([P, NB * H], F32, tag="S1")
            S2 = pp.tile([P, NB * H], F32, tag="S2")
            S3 = pp.tile([P, NB * H], F32, tag="S3")
            # small state tiles, packed into one allocation
            SMALL = pp.tile([P, NB * 12], F32, tag="SMALL")
            M = SMALL[:, 0 * NB:1 * NB]
            T1 = SMALL[:, 1 * NB:2 * NB]
            GA = SMALL[:, 2 * NB:3 * NB]   # scan scratch half 0
            GB = SMALL[:, 3 * NB:4 * NB]   # scan scratch half 1
            Q0 = SMALL[:, 4 * NB:5 * NB]
            A0 = SMALL[:, 5 * NB:6 * NB]
            KX = SMALL[:, 6 * NB:7 * NB]
            KY = SMALL[:, 7 * NB:8 * NB]
            W10 = SMALL[:, 8 * NB:9 * NB]
            RMU0 = SMALL[:, 9 * NB:10 * NB]
            MSK0 = SMALL[:, 10 * NB:11 * NB]

            # multi-dim views
            IS3 = IS.rearrange("p (b s) -> p b s", b=NB)
            RP3 = RP.rearrange("p (b k) -> p b k", b=NB)
            A23 = A2.rearrange("p (b t) -> p b t", b=NB)
            BQ3 = BQ.rearrange("p (b t) -> p b t", b=NB)
            C3 = CARR.rearrange("p (b t) -> p b t", b=NB)

            px0 = IS3[:, :, 0]
            py0 = IS3[:, :, 1]
            vx0 = IS3[:, :, 2]
            vy0 = IS3[:, :, 3]

            # ---------------- phase 0: loads + precompute ----------------
            sy.dma_start(out=IS[:], in_=ist_r[:])

            # actions -> A2, BQ, CARR (two 2MB chunks; squares on ScalarE)
            for hb in range(2):
                chunk = pp.tile([P, 8 * H * 2], F32, tag="S1" if hb == 0 else "CH1")
                for dq in range(2):
                    sy.dma_start(
                        out=chunk[:, dq * 2048:(dq + 1) * 2048],
                        in_=act_r[:, hb * 4096 + dq * 2048:hb * 4096 + (dq + 1) * 2048],
                    )
                ch = chunk.rearrange("p (b t a) -> p b t a", b=8, t=H)
                thr = ch[:, :, :, 0]
                tor = ch[:, :, :, 1]
                bsl = slice(hb * 8, (hb + 1) * 8)
                v.tensor_scalar(A23[:, bsl, :], thr, 2.0 * DT, None, Alu.mult)
                v.tensor_scalar(BQ3[:, bsl, :], tor, DT, None, Alu.mult)
                sq = pp.tile([P, 8 * H], F32, tag="S3")
                sq3 = sq.rearrange("p (b t) -> p b t", b=8)
                sc.activation(sq3, thr, AF.Square, scale=DT)   # (DT*T)^2
                sq2 = pp.tile([P, 8 * H], F32, tag="S1" if hb == 1 else "S2")
                sq23 = sq2.rearrange("p (b t) -> p b t", b=8)
                sc.activation(sq23, tor, AF.Square, scale=DT)  # (DT*Q)^2
                v.tensor_add(C3[:, bsl, :], sq3, sq23)

            # r0, m0
            sc.activation(GA, vx0, AF.Square)
            sc.activation(GB, vy0, AF.Square)
            v.tensor_add(M, GA, GB)                      # m0 = r0^2
            sc.activation(RP3[:, :, 0], M, AF.Sqrt)      # r0
            r0 = RP3[:, :, 0]

            # theta0/4 prep: w10 = r0 + vx0, rationalized to vy0^2/(r0 - vx0)
            # when vx0 < 0.  All reciprocals are deferred to the ln/exp table
            # section after the scan (no custom DVE ops available).
            v.tensor_add(W10, r0, vx0)                   # w10 direct
            v.tensor_sub(RMU0, r0, vx0)                  # r0 - vx0
            MSK0i = MSK0.bitcast(mybir.dt.int32)
            v.tensor_scalar(MSK0i, vx0, 0.0, None, Alu.is_lt)  # mask vx0 < 0

            # pos cumsum seeds
            v.scalar_tensor_tensor(KX, vx0, DT / 2.0, px0, Alu.mult, Alu.add)
            v.scalar_tensor_tensor(KY, vy0, DT / 2.0, py0, Alu.mult, Alu.add)

            # ---------------- phase 1: radius scan ----------------
            # m' = (m + c_t) + (2 a_t) * r_t ; r_{t+1} = sqrt(m')
            # two staggered halves so ScalarE sqrt overlaps VectorE updates
            halves = [slice(0, 8), slice(8, 16)]
            Mh = [M[:, 0:8], M[:, 8:16]]
            T1h = [T1[:, 0:8], T1[:, 8:16]]
            Gh = [GA[:, 0:8], GB[:, 0:8]]
            # Pre-allocate the first two output-chunk staging tiles so their
            # extras channels (which depend only on initial_state) can be
            # broadcast by gpsimd DURING the radius scan.  One extras op is
            # interleaved every 16 scan steps: gpsimd's T1 stream stays ahead
            # of the DVE/Act chain (scan consumes T1 at ~600ns/step; gpsimd
            # produces at ~340ns/step + 131ns/step of amortized extras).
            CB = 4                       # batch-columns per output chunk
            CW = CB * H
            OUTC_pre0 = op.tile([P, CB * H * S], F32, tag="OUTC")
            OUTC_pre1 = op.tile([P, CB * H * S], F32, tag="OUTC")
            OUTC_pre = [OUTC_pre0, OUTC_pre1]

            def emit_extras(ch, k, outc):
                out_ap = bass.AP(
                    outc.tensor, 4 + k, [[CB * H * S, P], [H * S, CB], [S, H]]
                )
                in_ap = bass.AP(
                    IS.tensor, ch * CB * S + 4 + k,
                    [[NB * S, P], [S, CB], [0, H]],
                )
                g.tensor_copy(out_ap, in_ap)

            # T1_t = m_t + c_t is computed at the END of iteration t-1 (just
            # after m_t is produced) so that on the in-order DVE queue it
            # executes during the wait for the sqrt semaphore instead of
            # delaying the chain-critical G/M' ops.
            for hf in (0, 1):
                v.tensor_add(T1h[hf], Mh[hf], C3[:, halves[hf], 0])
            for t in range(H):
                if t % 16 == 0 and 16 <= t <= 128:
                    idx = t // 16 - 1  # 0..7 -> (ch, k)
                    emit_extras(idx // 4, idx % 4, OUTC_pre[idx // 4])
                for hf in (0, 1):
                    v.tensor_mul(Gh[hf], A23[:, halves[hf], t], RP3[:, halves[hf], t])
                    v.tensor_add(Mh[hf], T1h[hf], Gh[hf])
                    sc.activation(RP3[:, halves[hf], t + 1], Mh[hf], AF.Sqrt)
                if t + 1 < H:
                    for hf in (0, 1):
                        v.tensor_add(T1h[hf], Mh[hf], C3[:, halves[hf], t + 1])

            # ---------------- phase 2: angles, velocities, positions ------
            Rsh = RP3[:, :, 0:H]     # r_t
            Rpo = RP3[:, :, 1:HP]    # r_{t+1}
            S1_3 = S1.rearrange("p (b t) -> p b t", b=NB)
            S2_3 = S2.rearrange("p (b t) -> p b t", b=NB)
            S3_3 = S3.rearrange("p (b t) -> p b t", b=NB)

            # theta0 chain (small; DVE reciprocal keeps Act in the sqrt table)
            v.reciprocal(GA, RMU0)                        # 1/(r0-vx0)
            v.tensor_mul(GB, vy0, GA)
            v.tensor_mul(GB, vy0, GB)                     # alt0
            v.copy_predicated(W10, MSK0i, GB)             # w10
            v.tensor_mul(GB, r0, W10)
            sc.activation(GB, GB, AF.Sqrt, scale=2.0)     # h0 = sqrt(2*w2)
            v.tensor_add(GB, GB, W10)                     # den0
            v.reciprocal(GA, GB)
            v.tensor_mul(Q0, vy0, GA)                     # q0

            # Phase 2 proper runs in TWO COLUMN HALVES so the first half's
            # output DMAs overlap the second half's compute.  Per half:
            # A-section (select-free atan2 identity):
            #   delta = sign(b) * (pi/2 - 2*atan(u / (r' + |b|))),  u = r_t + a
            # whose atan argument is always within [-1, 1] and well-conditioned
            # in every quadrant (no predicated rationalization); then trig via
            # magic-constant range reduction, then the C-section + DMA out.
            MAGIC = float(1.5 * 2 ** 23)
            INV_HPI = float(2.0 / np.pi)                  # turns = Theta*4/(2*pi)
            BQi = BQ[:].bitcast(mybir.dt.int32)
            HW4 = 4 * H                                   # floats per quarter
            VXY = pp.tile([P, 4 * CW], F32, tag="CH1")    # vxs/vys ping-pong
            for hs in range(4):
                colh = slice(hs * 4, (hs + 1) * 4)
                fsl = slice(hs * HW4, (hs + 1) * HW4)
                S1h, S2h, S3h = S1[:, fsl], S2[:, fsl], S3[:, fsl]
                S1h3 = S1_3[:, colh, :]
                S2h3 = S2_3[:, colh, :]
                S3h3 = S3_3[:, colh, :]
                A23h = A23[:, colh, :]
                Rsh_h = RP3[:, colh, 0:H]
                Rpo_h = RP3[:, colh, 1:HP]
                BQhi = BQ[:, fsl].bitcast(mybir.dt.int32)
                # A-section
                v.scalar_tensor_tensor(S1h3, A23h, 0.5, Rsh_h, Alu.mult, Alu.add)
                S2hi = S2h.bitcast(mybir.dt.int32)
                v.tensor_scalar(S2hi, BQhi, 0x7FFFFFFF, None, Alu.bitwise_and)
                v.tensor_add(S2h3, S2h3, Rpo_h)           # den = r' + |b|
                sc.activation(S3h, S2h, AF.Ln)
                sc.activation(S3h, S3h, AF.Exp, scale=-1.0)   # rden
                v.tensor_mul(S2h3, S1h3, S3h3)            # qu = u*rden
                # trig table phase; |qu| <= 1 + O(eps) needs no clamp
                if hs == 0:
                    sc.activation(A0, Q0, AF.Arctan)      # theta0/4
                sc.activation(S1h3, S2h3, AF.Arctan)      # At
                v.tensor_scalar(S1h, S1h, -0.5, PI / 8.0, Alu.mult, Alu.add)
                S1hi = S1h.bitcast(mybir.dt.int32)
                S3hi = S3h.bitcast(mybir.dt.int32)
                v.tensor_scalar(S3hi, BQhi, -0x80000000, None, Alu.bitwise_and)
                v.tensor_tensor(S1hi, S1hi, S3hi, Alu.bitwise_or)  # delta/4
                for b in range(4):
                    gb = hs * 4 + b
                    bs = slice(gb * H, (gb + 1) * H)
                    v.tensor_tensor_scan(
                        S3[:, bs], S1[:, bs], S1[:, bs],
                        initial=A0[:, gb:gb + 1], op0=Alu.add, op1=Alu.bypass,
                    )                                      # Theta
                v.tensor_scalar(S2h, S3h, INV_HPI, None, Alu.mult)       # yS
                v.tensor_scalar(S1h, S2h, MAGIC, -MAGIC, Alu.add, Alu.add)
                v.tensor_sub(S2h, S2h, S1h)               # fS
                sc.activation(S2h, S2h, AF.Sin, scale=TWO_PI)   # sin
                v.tensor_scalar(S1h, S3h, INV_HPI, 0.25, Alu.mult, Alu.add)
                v.tensor_scalar(S3h, S1h, MAGIC, -MAGIC, Alu.add, Alu.add)
                v.tensor_sub(S1h, S1h, S3h)               # fC
                sc.activation(S1h, S1h, AF.Sin, scale=TWO_PI)   # cos
                # C-section: one output chunk per quarter
                if True:
                    ch = hs
                    cols = slice(ch * CB, (ch + 1) * CB)
                    OUTC = OUTC_pre[ch] if ch < 2 else op.tile(
                        [P, CB * H * S], F32, tag="OUTC"
                    )
                    OC4 = OUTC.rearrange("p (b t s) -> p b t s", b=CB, t=H)
                    base = (hs % 2) * 2 * CW
                    vxs = VXY[:, base:base + CW]
                    vys = VXY[:, base + CW:base + 2 * CW]
                    vxs3 = vxs.rearrange("p (b t) -> p b t", b=CB)
                    vys3 = vys.rearrange("p (b t) -> p b t", b=CB)
                    Rpo_c = RP3[:, cols, 1:HP]
                    sin_c = S2_3[:, cols, :]
                    cos_c = S1_3[:, cols, :]
                    g.tensor_mul(OC4[:, :, :, 2], Rpo_c, cos_c)           # vx
                    g.tensor_mul(OC4[:, :, :, 3], Rpo_c, sin_c)           # vy
                    v.scalar_tensor_tensor(vxs3, cos_c, DT, Rpo_c, Alu.mult, Alu.mult)
                    v.scalar_tensor_tensor(vys3, sin_c, DT, Rpo_c, Alu.mult, Alu.mult)
                    for j in range(CB):
                        b = ch * CB + j
                        js = slice(j * H, (j + 1) * H)
                        v.tensor_tensor_scan(
                            OC4[:, j, :, 0], vxs[:, js], vxs[:, js],
                            initial=KX[:, b:b + 1], op0=Alu.add, op1=Alu.bypass,
                        )
                        v.tensor_tensor_scan(
                            OC4[:, j, :, 1], vys[:, js], vys[:, js],
                            initial=KY[:, b:b + 1], op0=Alu.add, op1=Alu.bypass,
                        )
                    v.scalar_tensor_tensor(
                        OC4[:, :, :, 0], vxs3, -0.5, OC4[:, :, :, 0], Alu.mult, Alu.add
                    )
                    v.scalar_tensor_tensor(
                        OC4[:, :, :, 1], vys3, -0.5, OC4[:, :, :, 1], Alu.mult, Alu.add
                    )
                    # extra columns broadcast from initial_state (gpsimd);
                    # chunks 0 and 1 were prefilled during the radius scan
                    if ch >= 2:
                        for k in range(4):
                            emit_extras(ch, k, OUTC)
                    hw = CB * H * S // 2
                    base_o = ch * CB * H * S
                    sy.dma_start(
                        out=traj_r[:, base_o:base_o + hw], in_=OUTC[:, 0:hw]
                    )
                    sy.dma_start(
                        out=traj_r[:, base_o + hw:base_o + 2 * hw],
                        in_=OUTC[:, hw:2 * hw],
                    )

    nc.finalize()
    if fixups:
        _split_multi_waits(nc)
    return nc


def _split_multi_waits(nc):
    """This toolchain's walrus embeds at most ONE sync-wait per instruction.
    Move all but the last wait of any multi-wait instruction onto NoOps
    inserted just before it (same engine, program order preserved).  Also
    drop the tail EVENT_SEMAPHORE_RANGE_CLEAR InstISA, whose raw encoding
    this walrus rejects ("ISA wrong length")."""
    n = 0
    for fn in nc.m.functions:
        for bb in fn.blocks:
            idx = 0
            while idx < len(bb.instructions):
                inst = bb.instructions[idx]
                if (
                    isinstance(inst, mybir.InstISA)
                    and getattr(inst, "op_name", "") == "EVENT_SEMAPHORE_RANGE_CLEAR"
                ):
                    del bb.instructions[idx]
                    continue
                si = getattr(inst, "sync_info", None)
                if si is not None and si.on_wait and len(si.on_wait) >= 2:
                    extra = list(si.on_wait[:-1])
                    keep = list(si.on_wait[-1:])
                    for w in extra:
                        nop = mybir.InstNoOp(
                            name=f"{inst.name}_wsplit{n}", ins=[], outs=[]
                        )
                        n += 1
                        nop.engine = inst.engine
                        nop.sync_info = mybir.SyncInfo(on_wait=[w], on_update=[])
                        bb.instructions.insert(idx, nop)
                        idx += 1
                    inst.sync_info = mybir.SyncInfo(
                        on_wait=keep, on_update=list(si.on_update)
                    )
                idx += 1
    return nc


def _get_built():
    global _BUILT
    if _BUILT is None:
        _BUILT = build_nc()
    return _BUILT


def kernel(initial_state: np.ndarray, actions: np.ndarray) -> np.ndarray:
    from concourse.bass_utils import run_bass_kernel_spmd

    nc = _get_built()
    in_maps = []
    for c in range(NCORES):
        sl = slice(c * BC, (c + 1) * BC)
        in_maps.append(
            {
                "initial_state": np.ascontiguousarray(initial_state[sl]),
                "actions": np.ascontiguousarray(actions[sl]),
            }
        )
    res = run_bass_kernel_spmd(nc, in_maps, core_ids=list(range(NCORES)))
    out = np.concatenate([r["traj"] for r in res.results], axis=0)
    return out

